# revision 6
# baseline (speedup 1.0000x reference)
"""CRF loss kernel v3 for Trainium2: fp8 one-hots + DoubleRow count matmuls.

Same layout as v2 (position = 256p + k, 257-column mtag with boundary col 0),
but the one-hot store h8 is a persistent [128, 257, 128] float8e4 tile:
  - emit mm (per tile k):   ps_e += h8[k+1]^T @ E_k    (fp8 lhsT x bf16 rhs)
  - count mm (per even k):  ps_c += DoubleRow(lhsT=h8[k+1:k+3], rhs=h8[k:k+2])
    -- one fp8 DoubleRow matmul counts TWO tile-pairs at 0.5 cyc/row,
    cutting count-matmul PE time 4x (verified exact on HW).
PE drops from 27.3k to ~17.1k ns; one-hot generation is spread across
three engines (group modes: AC = DVE bf16 one-hots + Act batch-convert
to fp8, PD = Pool direct fp8, DD = DVE direct fp8), leaving the kernel
paced by the 8MB/core bf16 emission DMA stream (~24.1k ns busy).
"""
import sys
import json

for p in ('/opt/trn_rl_repo', '/opt/trn_rl_repo/concourse'):
    if p not in sys.path:
        sys.path.insert(0, p)

import numpy as np

B, S, T = 512, 512, 128
NCORES = 8
BSH = B // NCORES
NPOS = BSH * S                 # 32768 positions per core
KT = NPOS // 128               # 256 tiles
G = 8
NEG = KT // G                  # 32 E groups
NHG = NEG + 1                  # 33 one-hot groups (257 one-hots)


def _mode(g):
    # DD: DVE direct fp8; PD: Pool direct fp8; AC: DVE bf16 + Act convert;
    # PC: DVE bf16 + Pool convert
    if g < 4:
        return 'DD'
    return ['PD', 'AC', 'AC', 'PD', 'AC', 'DD', 'AC'][(g - 4) % 7]


def _split_waits_json(bir_bytes: bytes, max_waits: int = 1) -> bytes:
    d = json.loads(bir_bytes)
    ctr = 0
    for f in d['functions']:
        for blk in f['blocks']:
            insts = blk.get('instructions')
            if not insts:
                continue
            out = []
            changed = False
            for ins in insts:
                si = ins.get('sync_info')
                if si and len(si.get('on_wait') or []) > max_waits:
                    waits = si['on_wait']
                    for w in waits[:-max_waits]:
                        ctr += 1
                        nop = {'engine': ins['engine'], 'ins': [], 'outs': [],
                               'name': f'wsplit-{ctr}', 'opcode': 'NoOp',
                               'sync_info': {'on_wait': [w], 'on_update': []}}
                        if 'debug' in ins:
                            nop['debug'] = ins['debug']
                        out.append(nop)
                    si['on_wait'] = waits[-max_waits:]
                    changed = True
                out.append(ins)
            if changed:
                blk['instructions'] = out
    return json.dumps(d).encode()


_patched = False


def _install_patch(bass_module):
    global _patched
    if _patched:
        return
    _patched = True
    orig = bass_module.Bass.to_json_bytes

    def patched(self):
        return _split_waits_json(orig(self))

    bass_module.Bass.to_json_bytes = patched


def _build():
    import concourse.bass as bass
    import concourse.mybir as mybir
    import concourse.tile as tile
    from concourse.masks import make_identity
    _install_patch(bass)
    f32 = mybir.dt.float32
    bf16 = mybir.dt.bfloat16
    fp8 = mybir.dt.float8e4
    i32 = mybir.dt.int32
    Alu = mybir.AluOpType
    DR = mybir.MatmulPerfMode.DoubleRow

    nc = bass.Bass()
    em = nc.dram_tensor('em', [NPOS, T], bf16, kind='ExternalInput')
    mtag = nc.dram_tensor('mtag', [128, KT + 1], bf16, kind='ExternalInput')
    tr = nc.dram_tensor('tr', [T, T], f32, kind='ExternalInput')
    out = nc.dram_tensor('out', [128, 3], f32, kind='ExternalOutput')

    em_r = em.rearrange("(p k) t -> p k t", k=KT)

    with tile.TileContext(nc) as tc:
        with tc.tile_pool(name='per', bufs=1) as per, \
             tc.tile_pool(name='eblk', bufs=5) as eblk, \
             tc.tile_pool(name='h16p', bufs=8) as h16p, \
             tc.tile_pool(name='ps', bufs=1, space='PSUM') as psp:

            # startup: one LONG first transfer (full E0) so the following
            # hwdge+dge latency chains hide under it; tags second
            e_first = eblk.tile([128, G, 128], bf16, tag='e')
            nc.sync.dma_start(out=e_first, in_=em_r[:, 0:G, :])
            mtag_b = per.tile([128, KT + 1], bf16)
            nc.sync.dma_start(out=mtag_b, in_=mtag[:, :])

            iota_i = per.tile([128, 128], i32)
            nc.gpsimd.iota(iota_i, pattern=[[1, 128]], base=0, channel_multiplier=0)
            iota_b = per.tile([128, 128], bf16)
            nc.vector.tensor_copy(iota_b, iota_i)
            mtag_sb = per.tile([128, KT + 1], f32)
            nc.vector.tensor_copy(mtag_sb, mtag_b)

            idt = per.tile([128, 256], f32)    # [identity | transitions^T]
            make_identity(nc, idt[:, 0:128])

            red = per.tile([128, 3], f32)
            msk_scr = per.tile([128, KT], bf16)
            scr = per.tile([128, 256], f32)

            h8 = per.tile([128, KT + 1, 128], fp8)   # persistent one-hot store

            ps_e = psp.tile([128, 128], f32)
            ps_c = psp.tile([128, 128], f32)

            next_ck = 0    # next even k whose DoubleRow count mm to issue
            e_tiles = {}
            LAG = 2        # groups of one-hot lookahead before matmuls consume
            for g in range(NHG + LAG):
                if g < NHG:
                    if g == 0:
                        e_tiles[0] = e_first
                    elif g < NEG and g % 2 == 1:
                        # E DMA groups of 16 tiles (2 one-hot groups)
                        ew = 2 if g + 1 < NEG else 1
                        e_blk = eblk.tile([128, 2 * G, 128], bf16, tag='e')
                        nc.sync.dma_start(out=e_blk[:, 0:ew * G, :],
                                          in_=em_r[:, g * G:(g + ew) * G, :])
                        e_tiles[g] = e_blk
                        if ew == 2:
                            e_tiles[g + 1] = None
                    if g == 3:
                        nc.sync.dma_start(out=idt[:, 128:256], in_=tr[:, :])
                    nt = G if g < NEG else 0
                    mode = _mode(g) if nt == G else 'DD'
                    if mode == 'DD':
                        idxs = list(range(g * G, g * G + nt)) + ([KT] if g == 0 else [])
                        for i in idxs:
                            nc.vector.tensor_scalar(out=h8[:, i, :], in0=iota_b,
                                                    scalar1=mtag_sb[:, i:i + 1],
                                                    scalar2=None, op0=Alu.is_equal)
                    elif mode in ('PD', 'HP'):
                        half = nt // 2 if mode == 'HP' else nt
                        for i in range(g * G, g * G + half):
                            nc.gpsimd.tensor_scalar(out=h8[:, i, :], in0=iota_b,
                                                    scalar1=mtag_sb[:, i:i + 1],
                                                    scalar2=None, op0=Alu.is_equal)
                        for i in range(g * G + half, g * G + nt):
                            nc.vector.tensor_scalar(out=h8[:, i, :], in0=iota_b,
                                                    scalar1=mtag_sb[:, i:i + 1],
                                                    scalar2=None, op0=Alu.is_equal)
                    else:   # AC / PC: DVE bf16 one-hots, then batch convert
                        h16 = h16p.tile([128, G, 128], bf16, tag='h16')
                        for j in range(nt):
                            i = g * G + j
                            nc.vector.tensor_scalar(out=h16[:, j, :], in0=iota_b,
                                                    scalar1=mtag_sb[:, i:i + 1],
                                                    scalar2=None, op0=Alu.is_equal)
                        dst = h8[:, g * G:g * G + G, :]
                        if mode == 'AC':
                            nc.scalar.copy(dst, h16)
                        else:
                            nc.gpsimd.tensor_copy(dst, h16)
                    if g == 4:
                        nc.vector.tensor_scalar(out=msk_scr,
                                                in0=mtag_sb[:, 1:KT + 1],
                                                scalar1=128.0, scalar2=0.0,
                                                op0=Alu.is_lt, op1=Alu.add,
                                                accum_out=red[:, 1:2])
                # matmuls trail the one-hot stream by LAG groups
                gm = g - LAG
                if gm < 0:
                    continue
                ntm = G if gm < NEG else 1
                for j in range(ntm):
                    k = gm * G + j - 1
                    if k < 0:
                        continue
                    eg = k // G
                    if eg == 0:
                        e_blkm, e_off = e_tiles[0], 0
                    elif e_tiles[eg] is None:
                        e_blkm, e_off = e_tiles[eg - 1], G
                    else:
                        e_blkm, e_off = e_tiles[eg], 0
                    nc.tensor.matmul(ps_e, lhsT=h8[:, k + 1, :],
                                     rhs=e_blkm[:, e_off + k % G, :],
                                     start=(k == 0), stop=(k == KT - 1),
                                     skip_group_check=True)
                hmax = gm * G + ntm - 1
                while next_ck + 2 <= hmax:
                    kk = next_ck
                    nc.tensor.matmul(ps_c, lhsT=h8[:, kk + 1:kk + 3, :],
                                     rhs=h8[:, kk:kk + 2, :],
                                     start=(kk == 0), stop=(kk == KT - 2),
                                     perf_mode=DR, skip_group_check=True)
                    next_ck += 2

            # ---- final reductions: count side first (its matmuls finish
            # during the last-E wait, so it overlaps the final emit mms) ----
            nc.vector.tensor_mul(scr[:, 128:256], ps_c, idt[:, 128:256])
            nc.vector.tensor_reduce(out=red[:, 2:3], in_=scr[:, 128:256],
                                    axis=mybir.AxisListType.X, op=Alu.add)
            nc.vector.tensor_mul(scr[:, 0:128], ps_e, idt[:, 0:128])
            nc.vector.tensor_reduce(out=red[:, 0:1], in_=scr[:, 0:128],
                                    axis=mybir.AxisListType.X, op=Alu.add)
            nc.sync.dma_start(out=out[:, :], in_=red)

    return nc


_nc_cache = None
last_results = None


def _prep_inputs(emissions, tags, mask, transitions):
    import ml_dtypes
    bf16 = ml_dtypes.bfloat16
    em_all = np.ascontiguousarray(emissions.reshape(B * S, T)).astype(bf16)
    tg = tags.reshape(B * S).astype(np.int32)
    mkb = mask.reshape(B * S).astype(np.int32)
    ft = (tg + 128 * (1 - mkb)).astype(bf16)
    trT = np.ascontiguousarray(transitions.astype(np.float32).T)

    p = np.arange(128)
    prevpos = 256 * p - 1
    in_maps = []
    for c in range(NCORES):
        lo = c * NPOS
        t_loc = tg[lo:lo + NPOS]
        m_loc = mkb[lo:lo + NPOS]
        valid = (p % 2 == 1) & (m_loc[prevpos] == 1)
        mt = np.empty((128, KT + 1), dtype=bf16)
        mt[:, 0] = np.where(valid, t_loc[prevpos], 128).astype(bf16)
        mt[:, 1:] = ft[lo:lo + NPOS].reshape(128, KT)
        in_maps.append({
            'em': np.ascontiguousarray(em_all[lo:lo + NPOS]),
            'mtag': mt,
            'tr': trT,
        })
    return in_maps


def kernel(emissions, tags, mask, transitions, _trace=False):
    global _nc_cache, last_results
    from concourse.bass_utils import run_bass_kernel_spmd
    if _nc_cache is None:
        _nc_cache = _build()
    nc = _nc_cache

    in_maps = _prep_inputs(emissions, tags, mask, transitions)
    res = run_bass_kernel_spmd(nc, in_maps, core_ids=list(range(NCORES)),
                               trace=_trace)
    last_results = res
    score = cnt = 0.0
    for r in res.results:
        v = np.asarray(r['out'], dtype=np.float64)
        score += v[:, 0].sum() + v[:, 2].sum()
        cnt += v[:, 1].sum()
    return np.float32(score / cnt)
